# revision 12
# baseline (speedup 1.0000x reference)
"""Expert-parallel MoE MLP kernel for Trainium2 (8 NeuronCores).

Problem: x[B=2,S=1024,H=1024] f32, expert_indices[B,S] int, 16 experts,
gate/up_proj[E,H,I], down_proj[E,I,H] (H=I=1024):
    out[n] = silu(x_n @ Wg[e_n]) * (x_n @ Wu[e_n]) @ Wd[e_n].T

Sharding: expert parallelism — core c owns experts {2c, 2c+1}. The host
groups tokens by expert (the "all-to-all dispatch" runs on host since the
kernel contract is full-input -> full-output), pads each expert's token
block to a fixed capacity, and each core runs dense per-expert GEMMs.

Device layout (per core, per expert e) keeps features on partitions so no
on-chip transposes are needed:
    xt    = X_e^T                [H=1024, P]
    Gt[i,n] = sum_h Wg[h,i]*xt[h,n];  inter = silu(Gt)*Ut
    Out^T[j,n] = sum_k WdT[k,j]*inter[k,n]   (WdT = Wd.T, host-transposed)

Matmuls run as float32r (TF32-like, 10 explicit HW mantissa bits,
~1.5e-4 rel error) at 1 cycle/row for moving dim >=256; all operands are
pre-rounded (unrounded f32r operands hard-fault the exec unit).

Perf structure:
  - weights are 24 MB/core of mandatory HBM traffic (the roofline); they
    are host-packed partition-major so each 1 MB DMA chunk moves 8 KB
    contiguous per-partition runs (near line-rate)
  - expert-0 gate weights + xt are interleaved per-chunk at the head of
    the HWDGE FIFO, and phases accumulate h-outer into 8 PSUM banks, so
    the first matmul starts after ~1 MB instead of ~6 MB
  - token blocks are DMA'd at their real (padded-to-16) width; the matmul
    still streams 256 columns, with the pad region zeroed once on-chip
  - each expert's outputs are staged in SBUF and shipped as one DMA so
    the kernel tail isn't serialized on 8 small-DMA dispatches
"""

import math

import numpy as np

E = 16
H = 1024
HT = 8          # H / 128 partition tiles
HB = 2          # h-blocks per weight DMA chunk (1 MB chunks)
N_CORES = 8
EPC = E // N_CORES  # experts per core
CW = 256        # matmul moving-dim width (>=256 keeps f32r at 1 cyc/row)

_NC_CACHE = {}


def _round_f32r(a, mant=16):
    """Round-to-nearest to `mant` explicit mantissa bits (f32r operand prep)."""
    xi = np.ascontiguousarray(a, dtype=np.float32).view(np.uint32).astype(np.uint64)
    drop = 23 - mant
    half = np.uint64(1 << (drop - 1))
    mask = np.uint64((~((1 << drop) - 1)) & 0xFFFFFFFF)
    return ((xi + half) & mask).astype(np.uint32).view(np.float32)


def _build_nc(ch: int, pio: int):
    """One SPMD program: EPC experts, ch chunks of CW token-slots per expert,
    pio real (DMA'd) token columns per expert, pio <= ch*CW."""
    import concourse.tile as tile
    from concourse import bacc, mybir
    from concourse.bass import ts

    f32 = mybir.dt.float32
    f32r = mybir.dt.float32r
    P = ch * CW
    NHB = HT // HB

    nc = bacc.Bacc("TRN2", target_bir_lowering=False, debug=False,
                   num_devices=N_CORES)
    # weights packed partition-major: w[e, proj, p, h, :] = Wproj[e][h*128+p, :]
    w = nc.dram_tensor("w", [EPC, 3, 128, HT, H], f32r, kind="ExternalInput")
    xt = nc.dram_tensor("xt", [EPC, 128, HT, P], f32r, kind="ExternalInput")
    out = nc.dram_tensor("out", [EPC, 128, HT, pio], f32, kind="ExternalOutput")

    wbufs = 4 if ch == 1 else 2
    with tile.TileContext(nc) as tc:
        with (
            tc.tile_pool(name="wp", bufs=wbufs) as wp,
            tc.tile_pool(name="xp", bufs=1) as xp,
            tc.tile_pool(name="gp", bufs=2) as gp,
            tc.tile_pool(name="ip", bufs=2) as ip,
            tc.tile_pool(name="op", bufs=2) as op,
            tc.tile_pool(name="ps", bufs=8, space="PSUM") as ps,
        ):
            x_sb = xp.tile([128, EPC, HT, P], f32r)

            # expert-0 gate weights and xt interleaved per-chunk at the head
            # of the HWDGE FIFO: the first matmul needs only chunk 0 of each.
            w0_sb = wp.tile([128, HT, H], f32r, tag="w", name="w0g")
            for hb in range(NHB):
                nc.sync.dma_start(
                    x_sb[:, 0, ts(hb, HB), :], xt[0, :, ts(hb, HB), :])
                nc.sync.dma_start(
                    w0_sb[:, ts(hb, HB), :], w[0, 0, :, ts(hb, HB), :])

            for e in range(EPC):
                w_sb = [w0_sb if (e, p_) == (0, 0) else
                        wp.tile([128, HT, H], f32r, tag="w", name=f"w{e}_{p_}")
                        for p_ in range(3)]
                for proj in range(3):
                    if (e, proj) == (0, 0):
                        continue
                    for hb in range(NHB):
                        nc.sync.dma_start(
                            w_sb[proj][:, ts(hb, HB), :],
                            w[e, proj, :, ts(hb, HB), :])
                if e + 1 < EPC:
                    nc.sync.dma_start(x_sb[:, e + 1, :, :], xt[e + 1])
                wg_sb, wu_sb, wd_sb = w_sb

                g_sb = gp.tile([128, HT, P], f32)       # silu(Gt)
                i_sb = ip.tile([128, HT, P], f32r)      # inter = silu(Gt)*Ut
                o_sb = op.tile([128, HT, pio], f32, tag="o")
                for c in range(ch):
                    cs = c * CW
                    # gate: h-outer accumulation into 8 PSUM banks
                    g_ps = [ps.tile([128, CW], f32, tag="ps", name=f"gps{i_}")
                            for i_ in range(HT)]
                    for h in range(HT):
                        for i in range(HT):
                            nc.tensor.matmul(
                                g_ps[i][:], wg_sb[:, h, ts(i, 128)],
                                x_sb[:, e, h, cs:cs + CW],
                                start=(h == 0), stop=(h == HT - 1))
                    for i in range(HT):
                        nc.scalar.activation(
                            g_sb[:, i, cs:cs + CW], g_ps[i][:],
                            mybir.ActivationFunctionType.Silu)
                    # up
                    u_ps = [ps.tile([128, CW], f32, tag="ps", name=f"ups{i_}")
                            for i_ in range(HT)]
                    for h in range(HT):
                        for i in range(HT):
                            nc.tensor.matmul(
                                u_ps[i][:], wu_sb[:, h, ts(i, 128)],
                                x_sb[:, e, h, cs:cs + CW],
                                start=(h == 0), stop=(h == HT - 1))
                    for i in range(HT):
                        nc.vector.tensor_mul(
                            i_sb[:, i, cs:cs + CW],
                            g_sb[:, i, cs:cs + CW], u_ps[i][:])
                    # down
                    o_ps = [ps.tile([128, CW], f32, tag="ps", name=f"ops{i_}")
                            for i_ in range(HT)]
                    for k in range(HT):
                        for j in range(HT):
                            nc.tensor.matmul(
                                o_ps[j][:], wd_sb[:, k, ts(j, 128)],
                                i_sb[:, k, cs:cs + CW],
                                start=(k == 0), stop=(k == HT - 1))
                    lo, hi = cs, min(cs + CW, pio)
                    for j in range(HT):
                        if hi > lo:
                            nc.vector.tensor_copy(
                                o_sb[:, j, lo:hi], o_ps[j][:, 0:hi - lo])
                # one output DMA per expert (staged, partition-major)
                nc.sync.dma_start(out[e], o_sb[:])
    nc.compile()
    return nc


def _get_nc(ch: int, pio: int):
    key = (ch, pio)
    if key not in _NC_CACHE:
        _NC_CACHE[key] = _build_nc(ch, pio)
    return _NC_CACHE[key]


def kernel(x, expert_indices, gate_proj, up_proj, down_proj):
    from concourse.bass_utils import run_bass_kernel_spmd

    x = np.ascontiguousarray(x, dtype=np.float32)
    gate_proj = np.ascontiguousarray(gate_proj, dtype=np.float32)
    up_proj = np.ascontiguousarray(up_proj, dtype=np.float32)
    down_proj = np.ascontiguousarray(down_proj, dtype=np.float32)
    b, s, h = x.shape
    assert (h, gate_proj.shape) == (H, (E, H, H)), (x.shape, gate_proj.shape)

    n = b * s
    xf = x.reshape(n, h)
    idx = np.asarray(expert_indices).reshape(n).astype(np.int64)

    order = np.argsort(idx, kind="stable")       # token ids grouped by expert
    counts = np.bincount(idx, minlength=E)
    starts = np.zeros(E + 1, dtype=np.int64)
    np.cumsum(counts, out=starts[1:])
    maxc = int(counts.max())
    ch = max(1, math.ceil(maxc / CW))
    P = ch * CW
    pio = min(P, max(16, 16 * math.ceil(maxc / 16)))

    # per-core inputs; weights packed partition-major [EPC,3,128,HT,H]
    wr = _round_f32r(
        np.stack([gate_proj, up_proj, down_proj.transpose(0, 2, 1)], axis=1)
    ).reshape(N_CORES, EPC, 3, HT, 128, H).transpose(0, 1, 2, 4, 3, 5)
    in_maps = []
    tok_ids = []
    for c in range(N_CORES):
        xt_c = np.zeros((EPC, H, P), dtype=np.float32)
        toks = []
        for le in range(EPC):
            e = c * EPC + le
            te = order[starts[e]:starts[e + 1]]
            toks.append(te)
            xt_c[le, :, :len(te)] = xf[te].T
        tok_ids.append(toks)
        in_maps.append({
            "w": np.ascontiguousarray(wr[c]),
            "xt": _round_f32r(xt_c).reshape(EPC, HT, 128, P)
                  .transpose(0, 2, 1, 3).copy(),
        })

    nc = _get_nc(ch, pio)
    res = run_bass_kernel_spmd(nc, in_maps, core_ids=list(range(N_CORES)))

    out = np.empty((n, h), dtype=np.float32)
    for c in range(N_CORES):
        o = res.results[c]["out"]                # [EPC, 128, HT, pio]
        for le in range(EPC):
            te = tok_ids[c][le]
            oe = o[le].transpose(1, 0, 2).reshape(h, pio)   # [H, pio]
            out[te] = oe[:, :len(te)].T
    return out.reshape(b, s, h)


# revision 13
# speedup vs baseline: 1.1223x; 1.1223x over previous
"""Expert-parallel MoE MLP kernel for Trainium2 (8 NeuronCores).

Problem: x[B=2,S=1024,H=1024] f32, expert_indices[B,S] int, 16 experts,
gate/up_proj[E,H,I], down_proj[E,I,H] (H=I=1024):
    out[n] = silu(x_n @ Wg[e_n]) * (x_n @ Wu[e_n]) @ Wd[e_n].T

Sharding: expert parallelism — core c owns experts {2c, 2c+1}. The host
groups tokens by expert (the "all-to-all dispatch" runs on host since the
kernel contract is full-input -> full-output), pads each expert's token
block to a fixed capacity, and each core runs dense per-expert GEMMs.

Device layout (per core, per expert e) keeps features on partitions so no
on-chip transposes are needed:
    xt    = X_e^T                [H=1024, P]
    Gt[i,n] = sum_h Wg[h,i]*xt[h,n];  inter = silu(Gt)*Ut
    Out^T[j,n] = sum_k WdT[k,j]*inter[k,n]   (WdT = Wd.T, host-transposed)

Matmuls run as float32r (TF32-like, 10 explicit HW mantissa bits,
~1.5e-4 rel error) at 1 cycle/row for moving dim >=256; all operands are
pre-rounded (unrounded f32r operands hard-fault the exec unit).

Perf structure:
  - weights are 24 MB/core of mandatory HBM traffic (the roofline); they
    are host-packed partition-major so each 1 MB DMA chunk moves 8 KB
    contiguous per-partition runs (near line-rate)
  - expert-0 gate weights + xt are interleaved per-chunk at the head of
    the HWDGE FIFO, and phases accumulate h-outer into 8 PSUM banks, so
    the first matmul starts after ~1 MB instead of ~6 MB
  - token blocks are DMA'd at their real (padded-to-16) width; the matmul
    still streams 256 columns, with the pad region zeroed once on-chip
  - each expert's outputs are staged in SBUF and shipped as one DMA so
    the kernel tail isn't serialized on 8 small-DMA dispatches
"""

import math

import numpy as np

E = 16
H = 1024
HT = 8          # H / 128 partition tiles
HB = 2          # h-blocks per weight DMA chunk (1 MB chunks)
N_CORES = 8
EPC = E // N_CORES  # experts per core
CW = 256        # matmul moving-dim width (>=256 keeps f32r at 1 cyc/row)

_NC_CACHE = {}


def _round_f32r(a, mant=16):
    """Round-to-nearest to `mant` explicit mantissa bits (f32r operand prep)."""
    xi = np.ascontiguousarray(a, dtype=np.float32).view(np.uint32).astype(np.uint64)
    drop = 23 - mant
    half = np.uint64(1 << (drop - 1))
    mask = np.uint64((~((1 << drop) - 1)) & 0xFFFFFFFF)
    return ((xi + half) & mask).astype(np.uint32).view(np.float32)


def _build_nc(ch: int, pio: int):
    """One SPMD program: EPC experts, ch chunks of CW token-slots per expert,
    pio real (DMA'd) token columns per expert, pio <= ch*CW."""
    import concourse.tile as tile
    from concourse import bacc, mybir
    from concourse.bass import ts

    f32 = mybir.dt.float32
    f32r = mybir.dt.float32r
    P = ch * CW
    NHB = HT // HB

    nc = bacc.Bacc("TRN2", target_bir_lowering=False, debug=False,
                   num_devices=N_CORES)
    # weights packed partition-major: w[e, proj, p, h, :] = Wproj[e][h*128+p, :]
    w = nc.dram_tensor("w", [EPC, 3, 128, HT, H], f32r, kind="ExternalInput")
    xt = nc.dram_tensor("xt", [EPC, 128, HT, pio], f32r, kind="ExternalInput")
    out = nc.dram_tensor("out", [EPC, 128, HT, pio], f32, kind="ExternalOutput")

    wbufs = 4 if ch == 1 else 2
    with tile.TileContext(nc) as tc:
        with (
            tc.tile_pool(name="wp", bufs=wbufs) as wp,
            tc.tile_pool(name="xp", bufs=1) as xp,
            tc.tile_pool(name="gp", bufs=2) as gp,
            tc.tile_pool(name="ip", bufs=2) as ip,
            tc.tile_pool(name="op", bufs=2) as op,
            tc.tile_pool(name="ps", bufs=8, space="PSUM") as ps,
        ):
            x_sb = xp.tile([128, EPC, HT, P], f32r)
            if pio < P:
                nc.vector.memset(x_sb[:, :, :, pio:P].bitcast(f32), 0.0)

            # expert-0 gate weights and xt interleaved per-chunk at the head
            # of the HWDGE FIFO: the first matmul needs only chunk 0 of each.
            w0_sb = wp.tile([128, HT, H], f32r, tag="w", name="w0g")
            for hb in range(NHB):
                nc.sync.dma_start(
                    x_sb[:, 0, ts(hb, HB), 0:pio], xt[0, :, ts(hb, HB), :])
                nc.sync.dma_start(
                    w0_sb[:, ts(hb, HB), :], w[0, 0, :, ts(hb, HB), :])

            for e in range(EPC):
                w_sb = [w0_sb if (e, p_) == (0, 0) else
                        wp.tile([128, HT, H], f32r, tag="w", name=f"w{e}_{p_}")
                        for p_ in range(3)]
                for proj in range(3):
                    if (e, proj) == (0, 0):
                        continue
                    for hb in range(NHB):
                        nc.sync.dma_start(
                            w_sb[proj][:, ts(hb, HB), :],
                            w[e, proj, :, ts(hb, HB), :])
                if e + 1 < EPC:
                    nc.sync.dma_start(x_sb[:, e + 1, :, 0:pio], xt[e + 1])
                wg_sb, wu_sb, wd_sb = w_sb

                g_sb = gp.tile([128, HT, P], f32)       # silu(Gt)
                i_sb = ip.tile([128, HT, P], f32r)      # inter = silu(Gt)*Ut
                o_sb = op.tile([128, HT, pio], f32, tag="o")
                for c in range(ch):
                    cs = c * CW
                    # gate: h-outer accumulation into 8 PSUM banks
                    g_ps = [ps.tile([128, CW], f32, tag="ps", name=f"gps{i_}")
                            for i_ in range(HT)]
                    for h in range(HT):
                        for i in range(HT):
                            nc.tensor.matmul(
                                g_ps[i][:], wg_sb[:, h, ts(i, 128)],
                                x_sb[:, e, h, cs:cs + CW],
                                start=(h == 0), stop=(h == HT - 1))
                    for i in range(HT):
                        nc.scalar.activation(
                            g_sb[:, i, cs:cs + CW], g_ps[i][:],
                            mybir.ActivationFunctionType.Silu)
                    # up
                    u_ps = [ps.tile([128, CW], f32, tag="ps", name=f"ups{i_}")
                            for i_ in range(HT)]
                    for h in range(HT):
                        for i in range(HT):
                            nc.tensor.matmul(
                                u_ps[i][:], wu_sb[:, h, ts(i, 128)],
                                x_sb[:, e, h, cs:cs + CW],
                                start=(h == 0), stop=(h == HT - 1))
                    for i in range(HT):
                        nc.vector.tensor_mul(
                            i_sb[:, i, cs:cs + CW],
                            g_sb[:, i, cs:cs + CW], u_ps[i][:])
                    # down
                    o_ps = [ps.tile([128, CW], f32, tag="ps", name=f"ops{i_}")
                            for i_ in range(HT)]
                    for k in range(HT):
                        for j in range(HT):
                            nc.tensor.matmul(
                                o_ps[j][:], wd_sb[:, k, ts(j, 128)],
                                i_sb[:, k, cs:cs + CW],
                                start=(k == 0), stop=(k == HT - 1))
                    lo, hi = cs, min(cs + CW, pio)
                    for j in range(HT):
                        if hi > lo:
                            nc.vector.tensor_copy(
                                o_sb[:, j, lo:hi], o_ps[j][:, 0:hi - lo])
                # one output DMA per expert (staged, partition-major)
                nc.sync.dma_start(out[e], o_sb[:])
    nc.compile()
    return nc


def _get_nc(ch: int, pio: int):
    key = (ch, pio)
    if key not in _NC_CACHE:
        _NC_CACHE[key] = _build_nc(ch, pio)
    return _NC_CACHE[key]


def kernel(x, expert_indices, gate_proj, up_proj, down_proj):
    from concourse.bass_utils import run_bass_kernel_spmd

    x = np.ascontiguousarray(x, dtype=np.float32)
    gate_proj = np.ascontiguousarray(gate_proj, dtype=np.float32)
    up_proj = np.ascontiguousarray(up_proj, dtype=np.float32)
    down_proj = np.ascontiguousarray(down_proj, dtype=np.float32)
    b, s, h = x.shape
    assert (h, gate_proj.shape) == (H, (E, H, H)), (x.shape, gate_proj.shape)

    n = b * s
    xf = x.reshape(n, h)
    idx = np.asarray(expert_indices).reshape(n).astype(np.int64)

    order = np.argsort(idx, kind="stable")       # token ids grouped by expert
    counts = np.bincount(idx, minlength=E)
    starts = np.zeros(E + 1, dtype=np.int64)
    np.cumsum(counts, out=starts[1:])
    maxc = int(counts.max())
    ch = max(1, math.ceil(maxc / CW))
    pio = min(ch * CW, max(16, 16 * math.ceil(maxc / 16)))

    # per-core inputs; weights packed partition-major [EPC,3,128,HT,H]
    wr = _round_f32r(
        np.stack([gate_proj, up_proj, down_proj.transpose(0, 2, 1)], axis=1)
    ).reshape(N_CORES, EPC, 3, HT, 128, H).transpose(0, 1, 2, 4, 3, 5)
    in_maps = []
    tok_ids = []
    for c in range(N_CORES):
        xt_c = np.zeros((EPC, H, pio), dtype=np.float32)
        toks = []
        for le in range(EPC):
            e = c * EPC + le
            te = order[starts[e]:starts[e + 1]]
            toks.append(te)
            xt_c[le, :, :len(te)] = xf[te].T
        tok_ids.append(toks)
        in_maps.append({
            "w": np.ascontiguousarray(wr[c]),
            "xt": _round_f32r(xt_c).reshape(EPC, HT, 128, pio)
                  .transpose(0, 2, 1, 3).copy(),
        })

    nc = _get_nc(ch, pio)
    res = run_bass_kernel_spmd(nc, in_maps, core_ids=list(range(N_CORES)))

    out = np.empty((n, h), dtype=np.float32)
    for c in range(N_CORES):
        o = res.results[c]["out"]                # [EPC, 128, HT, pio]
        for le in range(EPC):
            te = tok_ids[c][le]
            oe = o[le].transpose(1, 0, 2).reshape(h, pio)   # [H, pio]
            out[te] = oe[:, :len(te)].T
    return out.reshape(b, s, h)
